# revision 4
# baseline (speedup 1.0000x reference)
"""Trainium2 Bass kernel for nn_AttentionModule (segment_reduce).

Computation (per reference):
    wx   = features @ W
    s_g  = segment_sum(wx);  cnt_g = segment counts
    ctx  = tanh(s_g / max(cnt,1))            [G, D]
    score_n = <f_n, ctx[seg_n]>
    rep_g = segment_sum(score_n * f_n)       [G, D]

SPMD over 8 cores (segment-aligned node shard per core), one uniform
compiled program; all data-dependence flows through input tensors.

Per 4096-node chunk (32 tiles of 128 nodes + 1 boundary tile), software-
pipelined across chunks (loads 4 ahead, one-hot builds 2 ahead, rep pass
1 behind) so PE / DVE / ACT / DMA queues all stay busy:
  - one-hots built on DVE via is_equal (oh01 node x pair-slot from relp,
    m01 slot x window from t0rj, selT window x slot from a PE ones-
    broadcast of t0rj)
  - pass A: col-4-tiled PE matmuls (oh01 stationary M=32, f moving,
    concurrent via tile_position col groups) -> 32-wide pair-slot psum;
    merge to [d, w] via m01; ctx = tanh(recip * (merged.T @ W)) with the
    per-window mean folded into the ACT tanh scale -> ctx [w, d]
  - scatter ctx rows to slot columns via selT -> ctxs [d, (pt, slot)]
  - SPT: per tile, fT (host-transposed bf16 features) stationary x ctxs
    pair columns moving (N=32) -> SPT[n, j] = <f_n, ctx[pair-window j]>
  - ohsc = oh01 * SPT elementwise (oh01 is one-hot, so this IS the
    score-scaled one-hot; no reduce needed)
  - rep: col-4-tiled matmuls (ohsc stationary, f moving) -> slots,
    merged on device to [w, d] via m01, written bf16 to DRAM
  host: numpy pre/post-processing (index metadata, fT layout, assembly).
"""

import os
import sys
import math
from functools import lru_cache

for _p in ("/opt/trn_rl_repo", "/root/.axon_site/_ro/trn_rl_repo"):
    if os.path.isdir(_p) and _p not in sys.path:
        sys.path.insert(0, _p)

import numpy as np
import ml_dtypes

BF16 = ml_dtypes.bfloat16
FP8 = ml_dtypes.float8_e4m3

P = 128          # partitions
WINW = 128       # chunk segment-window width
PAIRW = 32       # pair segment-window width
NCORES = 8
MASK = -1000.0


def _ensure_ntff_hook():
    import types
    try:
        import antenv  # noqa
    except ImportError:
        return
    if "antenv.axon_hooks" in sys.modules:
        return
    hooks = types.ModuleType("antenv.axon_hooks")
    holder = [None]
    hooks.set_axon_ntff_profile_hook = lambda h: holder.__setitem__(0, h)
    hooks.get_axon_ntff_profile_hook = lambda: holder[0]
    sys.modules["antenv.axon_hooks"] = hooks
    import antenv
    antenv.axon_hooks = hooks
    try:
        from trn_agent_boot.trn_boot import _ntff_profile_via_ctypes
        so = "/opt/axon/libaxon_pjrt.so"
        if os.path.exists(so):
            hooks.set_axon_ntff_profile_hook(_ntff_profile_via_ctypes(so))
    except Exception:
        pass


@lru_cache(maxsize=4)
def build_program(nch: int, tpc: int = 32):
    import concourse.bass as bass  # noqa
    import concourse.mybir as mybir
    from concourse import bacc, tile

    tiles = tpc + 1
    n_pairs = tpc // 2
    n_pt = n_pairs * PAIRW // P        # slot groups (4 pairs of 32 each)
    f32 = mybir.dt.float32
    bf16 = mybir.dt.bfloat16
    fp8 = mybir.dt.float8e4

    nc = bacc.Bacc("TRN2", target_bir_lowering=False, debug=False,
                   num_devices=NCORES)

    # ---- DRAM I/O ----
    F_d = nc.dram_tensor("f_in", [nch, P, tiles, P], bf16,
                         kind="ExternalInput").ap()
    FT_d = nc.dram_tensor("ft_in", [nch, P, tiles, P], bf16,
                          kind="ExternalInput").ap()
    aux_d = nc.dram_tensor("aux", [nch, P, tiles + n_pt + 1], f32,
                           kind="ExternalInput").ap()
    t0rjs_d = nc.dram_tensor("t0rjs", [nch, 1, n_pt * P], bf16,
                             kind="ExternalInput").ap()
    ones_d = nc.dram_tensor("ones1", [1, P], bf16, kind="ExternalInput").ap()
    W_d = nc.dram_tensor("w_in", [P, P], bf16, kind="ExternalInput").ap()
    piota_d = nc.dram_tensor("piota", [P, 1], f32, kind="ExternalInput").ap()
    iotaw_d = nc.dram_tensor("iotaw", [P, P], bf16, kind="ExternalInput").ap()
    iotaj_d = nc.dram_tensor("iotaj", [P, tiles, PAIRW], bf16,
                             kind="ExternalInput").ap()
    rep_d = nc.dram_tensor("rep_out", [nch, P, P], bf16,
                           kind="ExternalOutput").ap()

    AluOp = mybir.AluOpType
    Act = mybir.ActivationFunctionType

    # pass A / rep slot-accumulation schedule (per pair region)
    def slot_base(t):
        u = (t - 1) // 2 if t >= 1 else 0
        return (u % 4) * PAIRW, u // 4

    order = [1, 0] + list(range(2, tiles))
    region_of = {t: ((t - 1) // 2 if t >= 1 else 0) for t in order}
    first_of_region = {}
    last_of_region = {}
    for t in order:
        u = region_of[t]
        first_of_region.setdefault(u, t)
        last_of_region[u] = t

    # SPT drain groups of 16 tiles
    gsz = 11
    n_grp = (tiles + gsz - 1) // gsz

    with tile.TileContext(nc) as tc:
        with tc.tile_pool(name="const", bufs=1) as cpool, \
             tc.tile_pool(name="fpool", bufs=6) as fpool, \
             tc.tile_pool(name="small", bufs=6) as spool, \
             tc.tile_pool(name="ps_s", bufs=2, space="PSUM") as ps_s_pool, \
             tc.tile_pool(name="ps_ctx", bufs=2, space="PSUM") as ps_ctx_pool, \
             tc.tile_pool(name="ps_spt", bufs=2, space="PSUM") as ps_spt_pool, \
             tc.tile_pool(name="ps_r", bufs=2, space="PSUM") as ps_r_pool:

            piota_t = cpool.tile([P, 1], f32)
            iotaw_t = cpool.tile([P, P], bf16)
            iotaj_t = cpool.tile([P, tiles, PAIRW], bf16)
            w_t = cpool.tile([P, P], bf16)
            ones_t = cpool.tile([1, P], bf16)
            nc.sync.dma_start(ones_t[:], ones_d[:])
            nc.sync.dma_start(piota_t[:], piota_d[:])
            nc.sync.dma_start(iotaw_t[:], iotaw_d[:])
            nc.sync.dma_start(iotaj_t[:], iotaj_d[:])
            nc.sync.dma_start(w_t[:], W_d[:])

            st = {}   # per-chunk live tiles

            # HAM warmup: keep PE busy while the first feature DMAs land
            ps_w = ps_ctx_pool.tile([P, P], f32, tag="ctx", name="ps_warm")
            for i in range(40):
                nc.tensor.matmul(ps_w[:], w_t[:], w_t[:], start=True,
                                 stop=True)

            for k in range(-4, nch + 2):
                # ---- 1a: big feature loads for chunk k+4 ----
                if 0 <= k + 4 < nch:
                    c = k + 4
                    s = {}
                    s["f"] = fpool.tile([P, tiles, P], bf16, tag="f", name=f"f_{c}")
                    nc.sync.dma_start(s["f"][:], F_d[c])
                    s["ft"] = fpool.tile([P, tiles, P], bf16, tag="ft", name=f"ft_{c}")
                    nc.sync.dma_start(s["ft"][:], FT_d[c])
                    st[c] = s

                # ---- 1b+2: aux loads + one-hot builds for chunk k+2 ----
                if 0 <= k + 2 < nch:
                    c = k + 2
                    s = st[c]
                    aux_t = spool.tile([P, tiles + n_pt + 1], f32,
                                       tag="aux", name=f"aux_{c}")
                    nc.gpsimd.dma_start(aux_t[:], aux_d[c])
                    relp_t = aux_t[:, 0:tiles]
                    t0rj_t = aux_t[:, tiles:tiles + n_pt]
                    s["recip"] = aux_t[:, tiles + n_pt:tiles + n_pt + 1]
                    t0rjs_t = spool.tile([1, n_pt * P], bf16, tag="t0rjs",
                                         name=f"t0rjs_{c}")
                    nc.gpsimd.dma_start(t0rjs_t[:], t0rjs_d[c])
                    ps_tb = ps_ctx_pool.tile([P, n_pt * P], f32, tag="ctx")
                    nc.tensor.matmul(ps_tb[:], ones_t[:], t0rjs_t[:],
                                     start=True, stop=True)
                    s["oh01"] = spool.tile([P, tiles, PAIRW], bf16, tag="oh01", name=f"oh01_{c}")
                    nc.vector.tensor_tensor(
                        out=s["oh01"][:],
                        in0=relp_t.unsqueeze(2).broadcast_to(
                            [P, tiles, PAIRW]),
                        in1=iotaj_t[:], op=AluOp.is_equal)
                    s["m01"] = spool.tile([P, n_pt, WINW], bf16, tag="m01", name=f"m01_{c}")
                    nc.vector.tensor_tensor(
                        out=s["m01"][:],
                        in0=t0rj_t.unsqueeze(2).broadcast_to(
                            [P, n_pt, WINW]),
                        in1=iotaw_t[:].unsqueeze(1).broadcast_to(
                            [P, n_pt, WINW]),
                        op=AluOp.is_equal)
                    s["selT"] = spool.tile([P, n_pt * P], bf16, tag="selT", name=f"selT_{c}")
                    nc.vector.tensor_scalar(
                        out=s["selT"][:], in0=ps_tb[:], scalar1=piota_t[:],
                        scalar2=0.0, op0=AluOp.subtract, op1=AluOp.is_equal)

                # ---- 3: merge(k) ----
                if 0 <= k < nch:
                    s = st[k]
                    ps_m = ps_ctx_pool.tile([P, P], f32, tag="ctx")
                    for pt in range(n_pt):
                        nc.tensor.matmul(ps_m[:], s["s_sb"][:, pt, :],
                                         s["m01"][:, pt, :],
                                         start=(pt == 0),
                                         stop=(pt == n_pt - 1))

                # ---- 4: rep(k-1) ----
                if 0 <= k - 1 < nch:
                    p = st[k - 1]
                    ps_r = ps_r_pool.tile([P, n_pt, P], f32, tag="ps_r",
                                          name=f"ps_r_{k - 1}")
                    for t in order:
                        base, pt = slot_base(t)
                        u = region_of[t]
                        nc.tensor.matmul(
                            ps_r[base:base + PAIRW, pt, :],
                            p["ohsc"][:, t, :], p["f"][:, t, :],
                            start=(first_of_region[u] == t),
                            stop=(last_of_region[u] == t),
                            tile_position=(0, base))

                # ---- 5+6: merged drain, pre(k) ----
                if 0 <= k < nch:
                    s = st[k]
                    m_sb = spool.tile([P, P], bf16, tag="mean",
                                      name=f"m_sb_{k}")
                    nc.scalar.copy(m_sb[:], ps_m[:])
                    ps_pre = ps_ctx_pool.tile([P, P], f32, tag="ctx")
                    nc.tensor.matmul(ps_pre[:], m_sb[:], w_t[:],
                                     start=True, stop=True)

                # ---- 9+10+11: ctx(k), scatter(k), ctx8(k) ----
                if 0 <= k < nch:
                    ctx_t = spool.tile([P, P], bf16, tag="ctx")
                    nc.scalar.activation(ctx_t[:], ps_pre[:], Act.Tanh,
                                         scale=s["recip"][:])
                    ps_cs = ps_ctx_pool.tile([P, n_pt * P], f32, tag="ctx")
                    nc.tensor.matmul(ps_cs[:], ctx_t[:], s["selT"][:],
                                     start=True, stop=True)
                    s["ctx8"] = spool.tile([P, n_pt, P], bf16, tag="ctx8", name=f"ctx8_{k}")
                    nc.scalar.copy(s["ctx8"][:], ps_cs[:].rearrange(
                        "p (a b) -> p a b", a=n_pt))

                # ---- 7+8: r_sb(k-1), repmerge(k-1) ----
                if 0 <= k - 1 < nch:
                    p["r_sb"] = spool.tile([P, n_pt, P], bf16, tag="r_sb",
                                           name=f"r_sb_{k - 1}")
                    h = n_pt // 2
                    nc.scalar.copy(p["r_sb"][:, :h, :], ps_r[:, :h, :])
                    ps_rm = ps_ctx_pool.tile([P, P], f32, tag="ctx")
                    for pt in range(h):
                        nc.tensor.matmul(ps_rm[:], p["m01"][:, pt, :],
                                         p["r_sb"][:, pt, :],
                                         start=(pt == 0), stop=False)
                    nc.scalar.copy(p["r_sb"][:, h:, :], ps_r[:, h:, :])
                    for pt in range(h, n_pt):
                        nc.tensor.matmul(ps_rm[:], p["m01"][:, pt, :],
                                         p["r_sb"][:, pt, :],
                                         start=False, stop=(pt == n_pt - 1))

                # ---- 12: store(k-1) ----
                if 0 <= k - 1 < nch:
                    rm_sb = spool.tile([P, P], bf16, tag="rm_sb",
                                      name=f"rm_sb_{k - 1}")
                    nc.scalar.copy(rm_sb[:], ps_rm[:])
                    nc.scalar.dma_start(rep_d[k - 1], rm_sb[:])
                    del st[k - 1]

                # ---- 13+14: passA(k+1), s_sb(k+1) ----
                if 0 <= k + 1 < nch:
                    s = st[k + 1]
                    ps_s = ps_s_pool.tile([P, n_pt, P], f32, tag="ps_s",
                                          name=f"ps_s_{k + 1}")
                    for t in order:
                        base, pt = slot_base(t)
                        u = region_of[t]
                        nc.tensor.matmul(
                            ps_s[base:base + PAIRW, pt, :],
                            s["oh01"][:, t, :], s["f"][:, t, :],
                            start=(first_of_region[u] == t),
                            stop=(last_of_region[u] == t),
                            tile_position=(0, base))
                    s["s_sb"] = spool.tile([P, n_pt, P], bf16, tag="s_sb", name=f"s_sb_{k + 1}")
                    nc.scalar.copy(s["s_sb"][:], ps_s[:])

                # ---- 15+16: SPT(k) + ohsc(k) ----
                if 0 <= k < nch:
                    s = st[k]
                    s["ohsc"] = spool.tile([P, tiles, PAIRW], bf16,
                                           tag="ohsc", name=f"ohsc_{k}")
                    for g in range(n_grp):
                        t0g = g * gsz
                        ng = min(gsz, tiles - t0g)
                        ps_spt = ps_spt_pool.tile([P, gsz, PAIRW], f32,
                                                  tag="spt")
                        for i in range(ng):
                            t = t0g + i
                            base, pt = slot_base(t)
                            nc.tensor.matmul(
                                ps_spt[:, i, :], s["ft"][:, t, :],
                                s["ctx8"][:, pt, base:base + PAIRW],
                                start=True, stop=True)
                        nc.vector.tensor_tensor(
                            out=s["ohsc"][:, t0g:t0g + ng, :],
                            in0=s["oh01"][:, t0g:t0g + ng, :],
                            in1=ps_spt[:, :ng, :], op=AluOp.mult)

    nc.compile()
    return nc


def host_prep(features, segment_ids, num_segments, weight_matrix, tpc=32,
              strict=True):
    """Numpy preprocessing. Returns (nch, in_maps, meta) or None if the
    geometry (window spans) doesn't fit for this tpc."""
    N, D = features.shape
    G = int(num_segments)
    seg = np.asarray(segment_ids).astype(np.int64)
    feats = np.asarray(features, dtype=np.float32)
    W = np.asarray(weight_matrix, dtype=np.float32)

    chunk = tpc * P
    tiles = tpc + 1
    nodes = tiles * P
    n_pairs = tpc // 2
    n_pt = n_pairs * PAIRW // P

    bnd = np.searchsorted(seg, np.arange(G + 1))
    cnt = np.diff(bnd)
    if cnt.max() > P:
        assert not strict, f"segment with {cnt.max()} nodes > {P}"
        return None
    recip_full = np.where(cnt > 0, 1.0 / np.maximum(cnt, 1), 0.0).astype(
        np.float32)

    cuts = [0]
    for c in range(1, NCORES):
        gidx = min(int(np.searchsorted(bnd, round(c * N / NCORES))), G)
        cuts.append(int(bnd[gidx]))
    cuts.append(N)
    counts = [cuts[c + 1] - cuts[c] for c in range(NCORES)]
    nch = max(1, math.ceil(max(counts) / chunk))

    in_maps = []
    meta = []
    for c in range(NCORES):
        n0, n1 = cuts[c], cuts[c + 1]
        Nc = n1 - n0
        segl = seg[n0:n1]

        f_pad = np.zeros((P + nch * chunk, D), np.float32)
        f_pad[P:P + Nc] = feats[n0:n1]
        fw = np.lib.stride_tricks.sliding_window_view(
            f_pad, (nodes, D))[::chunk, 0][:nch]          # [nch, nodes, D]
        f_in = np.ascontiguousarray(
            fw.reshape(nch, tiles, P, D).transpose(0, 2, 1, 3)).astype(BF16)
        ft = np.ascontiguousarray(
            fw.transpose(0, 2, 1)).reshape(nch, D, tiles, P).astype(BF16)

        v = np.arange(Nc)
        chunk_of = v // chunk
        g_lo, g_hi = int(segl[0]), int(segl[-1]) + 1
        own = (bnd[np.arange(g_lo, g_hi) + 1] - 1 - n0) // chunk
        own_of_node = own[segl - g_lo]
        valid = own_of_node == chunk_of

        pw = np.full((nch, n_pairs), 0, np.int64)
        for k in range(nch):
            for u in range(n_pairs):
                i = k * chunk + u * 2 * P
                pw[k, u] = segl[min(i, Nc - 1)]
        wk = pw[:, 0]

        relp = np.where(valid, segl - pw[chunk_of, ((v % chunk) // P) // 2],
                        MASK).astype(np.float32)

        rel32 = np.full((nch, P, tiles), MASK, np.float32)
        pad = np.full(nch * chunk - Nc, MASK, np.float32)
        rp = np.concatenate([relp, pad]).reshape(nch, tpc, P)
        rel32[:, :, 1:] = rp.transpose(0, 2, 1)

        for k in range(1, nch):
            lo = k * chunk - P
            if lo >= Nc:
                continue
            hi = min(k * chunk, Nc)
            idx = np.arange(lo, hi)
            bvalid = own_of_node[idx] == k
            br = np.where(bvalid, segl[idx] - wk[k], MASK).astype(np.float32)
            rel32[k, :hi - lo, 0] = br

        # geometry checks (fall back to smaller tpc on overflow)
        rel_ok = rel32[rel32 > MASK / 2]
        bad = (rel_ok.size and (rel_ok.min() < 0 or rel_ok.max() >= PAIRW)) \
            or (pw - wk[:, None]).max() + PAIRW > WINW
        if bad:
            assert not strict, "window overflow"
            return None

        # t0rj[k, s, pt] = pw[k, 4*pt + s//32] - wk[k] + s%32
        sl = np.arange(P)
        upt = 4 * np.arange(n_pt)[None, :] + (sl // PAIRW)[:, None]  # [P,n_pt]
        t0rj = (pw[:, upt] - wk[:, None, None]
                + (sl % PAIRW)[None, :, None]).astype(np.float32)
        t0rjs = t0rj.transpose(0, 2, 1).reshape(nch, 1, n_pt * P).astype(BF16)

        gw = wk[:, None] + np.arange(WINW)[None, :]
        recipcol = np.where(gw < G, recip_full[np.minimum(gw, G - 1)], 0.0
                            ).astype(np.float32).reshape(nch, P, 1)

        aux = np.concatenate([rel32, t0rj, recipcol], axis=2).astype(
            np.float32)
        in_maps.append({
            "f_in": f_in,
            "ft_in": ft,
            "aux": aux,
            "t0rjs": t0rjs,
            "w_in": W.astype(BF16),
            "piota": np.arange(P, dtype=np.float32)[:, None],
            "ones1": np.ones((1, P), BF16),
            "iotaw": np.broadcast_to(
                np.arange(P, dtype=np.float32)[None, :],
                (P, P)).astype(BF16).copy(),
            "iotaj": np.broadcast_to(
                np.arange(PAIRW, dtype=np.float32)[None, None, :],
                (P, tiles, PAIRW)).astype(BF16).copy(),
        })
        meta.append({"wk": wk, "tpc": tpc,
                     "gtgt": (t0rj + wk[:, None, None]).astype(np.int64)})
    return nch, in_maps, meta


def assemble(results, meta, G, D):
    rep = np.zeros((G, D), np.float32)
    for c in range(NCORES):
        out = np.asarray(results[c]["rep_out"], dtype=np.float32)
        m = meta[c]
        wk = m["wk"]
        nch = out.shape[0]
        for k in range(nch):
            w0 = int(wk[k])
            hi = min(G - w0, WINW)
            if hi > 0:
                rep[w0:w0 + hi] += out[k, :hi]
    return rep


_LAST_RUN = {}


def kernel(features, segment_ids, num_segments, weight_matrix):
    from concourse.bass_utils import run_bass_kernel_spmd
    _ensure_ntff_hook()

    G = int(num_segments)
    D = features.shape[1]
    tpc = 32
    prep = host_prep(features, segment_ids, num_segments, weight_matrix,
                     tpc=32, strict=False)
    if prep is None:
        tpc = 16
        prep = host_prep(features, segment_ids, num_segments, weight_matrix,
                         tpc=16, strict=True)
    nch, in_maps, meta = prep
    nc = build_program(nch, tpc)
    trace = bool(int(os.environ.get("BASS_KERNEL_TRACE", "0")))
    kw = {}
    if trace:
        kw["trace"] = True
        kw["tmpdir"] = os.environ.get("BASS_KERNEL_TRACE_DIR") or None
    res = run_bass_kernel_spmd(nc, in_maps, core_ids=list(range(NCORES)), **kw)
    _LAST_RUN["exec_time_ns"] = res.exec_time_ns
    _LAST_RUN["res"] = res
    return assemble(res.results, meta, G, D)
